# revision 12
# baseline (speedup 1.0000x reference)
"""Trainium2 Bass kernel for nn_CustomNetwork_83683142795560 (range-loss MLP).

Strategy (8 NeuronCores, SPMD single NEFF):
  - Host sorts the batch by class label (stable argsort of `target`) so each
    class occupies a contiguous row range; class boundaries become
    compile-time constants baked into the kernel.
  - Data-parallel MLP: each core gets 1024 sorted rows of x, computes the
    3-layer MLP in transposed layout (features on partitions), producing
    yT [10, 1024].
  - Pairwise stage via one augmented matmul: d2_eff[i,j] = u_i . v_j with
    u = [-2y; 1; sq_i - BIG; SB*onehot], v = [y; sq_j; 1; SB*onehot] (K=22),
    which equals |y_i - y_j|^2 - BIG*(label_i != label_j).  Strict-upper
    triangle handled by a per-row-tile skip (columns below the 128-aligned
    diagonal) plus one 128x128 additive penalty on the diagonal strip.
  - v-aug features are AllGathered; each core processes its own 1024 rows
    against a window of columns [row_block, row_block + 1024 + EXT) where
    EXT covers the largest class (dynamic-offset DMA by partition id).
  - Per-row top-k via the DVE Max8 instruction; per-row candidates are
    AllGathered, and every core replicates the tiny per-class top-k + centers
    + loss epilogue.  Host returns y (un-permuted) and core 0's loss.
"""

import sys

sys.path.insert(0, "/opt/trn_rl_repo")

import numpy as np

import concourse.bacc as bacc
import concourse.mybir as mybir
from concourse.bass import ds
from concourse.bass_utils import run_bass_kernel_spmd
from concourse.tile import TileContext

F32 = mybir.dt.float32
F32R = mybir.dt.float32r

NCORES = 8
B = 8192
NCLS = 10
BLK = B // NCORES  # rows per core
NT = BLK // 128  # 128-row tiles per core
D_IN, D_H1, D_H2, D_OUT = 784, 500, 256, 10
BIGP = 16384.0  # class-mismatch penalty (power of two, exact in f32/f32r)
SBP = 128.0  # sqrt(BIGP)
KAUG = 23  # 10 (y) + [1, sq] + 10 (onehot) + [-BIG const]

# dtype of the big matmuls. f32r streams 1 col/cycle (vs 4 for f32) at
# ~1e-4 relative error; flip to F32 if accuracy requires.
DT_MLP = F32R
DT_PW = F32R

_BUILD_CACHE: dict = {}

# test harness hooks: set TRACE=True before calling kernel() to profile;
# exec time lands in LAST_PERF.
TRACE = False
TRACE_CORES = None  # e.g. list(range(8))
LAST_PERF: dict = {}


def _ceil_to(x, m):
    return -(-x // m) * m


def _build(counts_key, k):
    """Trace + compile the SPMD Bass program for given class counts/topk."""
    counts = np.array(counts_key, dtype=np.int64)
    assert counts.sum() == B and len(counts) == NCLS
    bounds = np.concatenate([[0], np.cumsum(counts)])  # class c: [bounds[c], bounds[c+1])
    max_cls = int(counts.max())
    EXT = max(128, _ceil_to(max_cls, 128))
    NEXTRA = -(-EXT // BLK)  # extra rank blocks the window can reach into
    KP = int(k)
    assert 1 <= KP <= 8

    nc = bacc.Bacc("TRN2", target_bir_lowering=False, num_devices=NCORES)

    # ---- inputs ----
    X = nc.dram_tensor("X", [BLK, D_IN], F32, kind="ExternalInput")
    W1T = nc.dram_tensor("W1T", [D_IN, D_H1], DT_MLP, kind="ExternalInput")
    B1T = nc.dram_tensor("B1T", [D_H1], F32, kind="ExternalInput")
    W2T = nc.dram_tensor("W2T", [D_H1, D_H2], DT_MLP, kind="ExternalInput")
    B2T = nc.dram_tensor("B2T", [D_H2], F32, kind="ExternalInput")
    W3T = nc.dram_tensor("W3T", [D_H2, D_OUT], DT_MLP, kind="ExternalInput")
    B3T = nc.dram_tensor("B3T", [D_OUT], F32, kind="ExternalInput")
    OHB = nc.dram_tensor("OHB", [NCLS, BLK], DT_PW, kind="ExternalInput")
    TRI = nc.dram_tensor("TRI", [128, 128], F32, kind="ExternalInput")
    CTRI = nc.dram_tensor("CTRI", [NCLS, NCLS], F32, kind="ExternalInput")
    INVC = nc.dram_tensor("INVC", [NCLS, NCLS], F32, kind="ExternalInput")
    IDENT = nc.dram_tensor("IDENT", [128, 128], F32, kind="ExternalInput")
    CROWS = nc.dram_tensor("CROWS", [2, BLK], DT_PW, kind="ExternalInput")
    ASC = nc.dram_tensor("ASC", [1, 1], F32, kind="ExternalInput")
    MSC = nc.dram_tensor("MSC", [1, 1], F32, kind="ExternalInput")

    YOUT = nc.dram_tensor("YOUT", [BLK, D_OUT], F32, kind="ExternalOutput")
    DBG = nc.dram_tensor("DBG", [NCLS, 32], F32, kind="ExternalOutput")
    LOSS = nc.dram_tensor("LOSS", [1, 1], F32, kind="ExternalOutput")

    KC1 = 7  # k-chunks layer 1 (7 x 112 = 784)
    CK1 = 112
    M1 = 4  # m-tiles layer 1 (4 x 125 = 500)
    CM1 = 125
    KC2 = 4  # k-chunks layer 2 (4 x 125)
    M2T = 2  # m-tiles layer 2 (2 x 128 = 256)
    KC3 = 2  # k-chunks layer 3 (2 x 128)
    NH = BLK // 512  # 512-wide batch halves

    with TileContext(nc) as tc:
        with (
            tc.tile_pool(name="const", bufs=1) as cpool,
            tc.tile_pool(name="persist", bufs=1) as pp,
            tc.tile_pool(name="dram", bufs=1, space="DRAM") as dp,
        ):
            ident = cpool.tile([128, 128], F32)
            nc.sync.dma_start(ident[:], IDENT[:])
            tri = cpool.tile([128, 128], F32)
            nc.sync.dma_start(tri[:], TRI[:])

            # persistent SBUF
            yT = pp.tile([D_OUT, BLK], F32)
            AU = pp.tile([KAUG, BLK], DT_PW)
            AV = pp.tile([KAUG, BLK], DT_PW)

            # DRAM comm buffers
            avd = dp.tile([KAUG, BLK], DT_PW)
            agout = dp.tile([NCORES + NEXTRA, KAUG, BLK], DT_PW)
            ag2in = dp.tile([KP, BLK], F32)
            ag2out = dp.tile([NCORES, KP, BLK], F32)
            bounce = dp.tile([KP, 8 * NCLS], F32)
            cbounce = dp.tile([1, NCLS], F32)

            # ---------------- phase A: MLP ----------------
            with (
                tc.tile_pool(name="mlp", bufs=1) as mp,
                tc.tile_pool(name="xload", bufs=3) as xp,
                tc.tile_pool(name="psA", bufs=2, space="PSUM") as psA,
                tc.tile_pool(name="psB", bufs=3, space="PSUM") as psB,
            ):
                # weights to SBUF
                w1 = [mp.tile([CK1, D_H1], DT_MLP, tag=f"w1_{i}", name=f"w1_{i}") for i in range(KC1)]
                for i in range(KC1):
                    nc.sync.dma_start(w1[i][:], W1T[i * CK1:(i + 1) * CK1, :])
                w2 = [mp.tile([CM1, D_H2], DT_MLP, tag=f"w2_{i}", name=f"w2_{i}") for i in range(KC2)]
                for i in range(KC2):
                    nc.sync.dma_start(w2[i][:], W2T[i * CM1:(i + 1) * CM1, :])
                w3 = [mp.tile([128, D_OUT], DT_MLP, tag=f"w3_{i}", name=f"w3_{i}") for i in range(KC3)]
                for i in range(KC3):
                    nc.sync.dma_start(w3[i][:], W3T[i * 128:(i + 1) * 128, :])
                b1 = [mp.tile([CM1, 1], F32, tag=f"b1_{m}", name=f"b1_{m}") for m in range(M1)]
                for m in range(M1):
                    nc.sync.dma_start(
                        b1[m][:],
                        B1T[m * CM1:(m + 1) * CM1].rearrange("(p a) -> p a", a=1),
                    )
                b2 = [mp.tile([128, 1], F32, tag=f"b2_{m}", name=f"b2_{m}") for m in range(M2T)]
                for m in range(M2T):
                    nc.sync.dma_start(
                        b2[m][:],
                        B2T[m * 128:(m + 1) * 128].rearrange("(p a) -> p a", a=1),
                    )
                b3 = mp.tile([D_OUT, 1], F32, tag="b3")
                nc.sync.dma_start(b3[:], B3T[:].rearrange("(p a) -> p a", a=1))

                # x -> xT (PE transpose, 112-wide feature chunks)
                xT = [mp.tile([CK1, BLK], DT_MLP, tag=f"xT_{i}", name=f"xT_{i}") for i in range(KC1)]
                for bt in range(NT):
                    xt = xp.tile([128, D_IN], F32, tag="x")
                    nc.sync.dma_start(xt[:], X[bt * 128:(bt + 1) * 128, :])
                    for f in range(KC1):
                        pt = psA.tile([CK1, 128], F32, tag="ptr")
                        nc.tensor.transpose(
                            pt[:], xt[:, f * CK1:(f + 1) * CK1], ident[:]
                        )
                        nc.scalar.copy(xT[f][:, bt * 128:(bt + 1) * 128], pt[:])

                # layer 1: h1T[m] = relu(W1[:, m].T @ x.T + b1[m])
                h1 = [mp.tile([CM1, BLK], DT_MLP, tag=f"h1_{m}", name=f"h1_{m}") for m in range(M1)]
                for m in range(M1):
                    for n in range(NH):
                        ps = psB.tile([128, 512], F32, tag="mm", name="ps_mm")[0:CM1, :]
                        for kk in range(KC1):
                            nc.tensor.matmul(
                                ps[:],
                                w1[kk][:, m * CM1:(m + 1) * CM1],
                                xT[kk][:, n * 512:(n + 1) * 512],
                                start=(kk == 0),
                                stop=(kk == KC1 - 1),
                            )
                        nc.scalar.activation(
                            h1[m][:, n * 512:(n + 1) * 512], ps[:],
                            mybir.ActivationFunctionType.Relu,
                            bias=b1[m][:], scale=1.0,
                        )

                # layer 2
                h2 = [mp.tile([128, BLK], DT_MLP, tag=f"h2_{m}", name=f"h2_{m}") for m in range(M2T)]
                for m in range(M2T):
                    for n in range(NH):
                        ps = psB.tile([128, 512], F32, tag="mm", name="ps_mm")
                        for kk in range(KC2):
                            nc.tensor.matmul(
                                ps[:],
                                w2[kk][:, m * 128:(m + 1) * 128],
                                h1[kk][:, n * 512:(n + 1) * 512],
                                start=(kk == 0),
                                stop=(kk == KC2 - 1),
                            )
                        nc.scalar.activation(
                            h2[m][:, n * 512:(n + 1) * 512], ps[:],
                            mybir.ActivationFunctionType.Relu,
                            bias=b2[m][:], scale=1.0,
                        )

                # layer 3 -> yT [10, BLK] (f32)
                for n in range(NH):
                    ps = psB.tile([128, 512], F32, tag="mm", name="ps_mm")[0:D_OUT, :]
                    for kk in range(KC3):
                        nc.tensor.matmul(
                            ps[:],
                            w3[kk][:],
                            h2[kk][:, n * 512:(n + 1) * 512],
                            start=(kk == 0),
                            stop=(kk == KC3 - 1),
                        )
                    nc.scalar.activation(
                        yT[:, n * 512:(n + 1) * 512], ps[:],
                        mybir.ActivationFunctionType.Relu,
                        bias=b3[:], scale=1.0,
                    )

                # y output: transpose yT back to [BLK, 10]
                for bt in range(NT):
                    ps = psA.tile([128, D_OUT], F32, tag="pty")
                    nc.tensor.transpose(
                        ps[:], yT[:, bt * 128:(bt + 1) * 128], ident[0:10, 0:10]
                    )
                    yn = xp.tile([128, D_OUT], F32, tag="ynat")
                    nc.scalar.copy(yn[:], ps[:])
                    nc.sync.dma_start(YOUT[bt * 128:(bt + 1) * 128, :], yn[:])

                # ---- augmented features ----
                ysq = mp.tile([D_OUT, BLK], F32, tag="ysq")
                nc.vector.tensor_mul(ysq[:], yT[:], yT[:])
                ones10 = mp.tile([D_OUT, 1], F32, tag="ones10")
                nc.vector.memset(ones10[:], 1.0)
                sq = mp.tile([1, BLK], F32, tag="sq")
                for n in range(NH):
                    ps = psB.tile([128, 512], F32, tag="mm", name="ps_mm")[0:1, :]
                    nc.tensor.matmul(
                        ps[:], ones10[:], ysq[:, n * 512:(n + 1) * 512],
                        start=True, stop=True,
                    )
                    nc.scalar.copy(sq[:, n * 512:(n + 1) * 512], ps[:])

                # u = [-2y; 1; sq_i; SB*oh; -BIG], v = [y; sq_j; 1; SB*oh; 1]
                # (compute engines may only address partitions at 32-aligned
                # bases, so off-aligned rows are assembled with DMAs)
                nc.vector.tensor_scalar_mul(AU[0:10, :], yT[:], -2.0)
                nc.sync.dma_start(AU[10:11, :], CROWS[0:1, :])
                nc.sync.dma_start(AU[11:12, :], sq[:].bitcast(DT_PW))
                nc.sync.dma_start(AU[12:22, :], OHB[:])
                nc.sync.dma_start(AU[22:23, :], CROWS[1:2, :])
                nc.vector.tensor_copy(AV[0:10, :], yT[:])
                nc.sync.dma_start(AV[10:11, :], sq[:].bitcast(DT_PW))
                nc.sync.dma_start(AV[11:12, :], CROWS[0:1, :])
                nc.sync.dma_start(AV[12:22, :], OHB[:])
                nc.sync.dma_start(AV[22:23, :], CROWS[0:1, :])

            # ---------------- AllGather #1 ----------------
            zt = pp.tile([KAUG, BLK], F32)
            nc.vector.memset(zt[:], 0.0)
            for e in range(NEXTRA):
                nc.sync.dma_start(
                    agout[NCORES + e:NCORES + e + 1, :, :].rearrange(
                        "a p n -> (a p) n"
                    ),
                    zt[:].bitcast(DT_PW),
                )
            nc.sync.dma_start(avd[:], AV[:])
            nc.gpsimd.collective_compute(
                "AllGather",
                mybir.AluOpType.bypass,
                replica_groups=[list(range(NCORES))],
                ins=[avd.opt()],
                outs=[agout[0:NCORES, :, :].opt()],
            )

            # window extension: columns [BLK, BLK+EXT) relative to own block
            pid = nc.gpsimd.partition_id()
            wext = pp.tile([KAUG, EXT], DT_PW)
            for e in range(NEXTRA):
                we = min(BLK, EXT - e * BLK)
                nc.gpsimd.dma_start(
                    wext[:, e * BLK:e * BLK + we],
                    agout[ds(pid + 1 + e, 1), :, 0:we].rearrange(
                        "a p n -> (a p) n"
                    ),
                )

            # ---------------- pairwise + per-row top-k ----------------
            P2 = pp.tile([128, 8 * KP], F32)
            with (
                tc.tile_pool(name="pw", bufs=3) as pwp,
                tc.tile_pool(name="psP", bufs=4, space="PSUM") as psP,
            ):
                for t in range(NT):
                    wt = BLK + EXT - t * 128
                    rb = pwp.tile([128, BLK + EXT], F32, tag="rowbuf")
                    lhs = AU[:, t * 128:(t + 1) * 128]
                    # segments: (src_ap, width, rowbuf offset)
                    segs = []
                    c0 = t * 128
                    while c0 < BLK:
                        w = min(512, BLK - c0)
                        segs.append((AV[:, c0:c0 + w], w, c0 - t * 128))
                        c0 += w
                    c0 = 0
                    while c0 < EXT:
                        w = min(512, EXT - c0)
                        segs.append(
                            (wext[:, c0:c0 + w], w, BLK - t * 128 + c0)
                        )
                        c0 += w
                    for si, (src, w, ro) in enumerate(segs):
                        ps = psP.tile([128, 512], F32, tag="pwps")
                        nc.tensor.matmul(
                            ps[:, 0:w], lhs, src, start=True, stop=True
                        )
                        if si == 0:
                            # diagonal strip penalty on first 128 cols
                            nc.vector.tensor_add(
                                rb[:, ro:ro + 128], ps[:, 0:128], tri[:]
                            )
                            if w > 128:
                                nc.scalar.copy(
                                    rb[:, ro + 128:ro + w], ps[:, 128:w]
                                )
                        else:
                            nc.scalar.copy(rb[:, ro:ro + w], ps[:, 0:w])
                    m8 = pwp.tile([128, 8], F32, tag="m8")
                    nc.vector.max(m8[:], rb[:, 0:wt])
                    for v in range(KP):
                        nc.vector.tensor_copy(
                            P2[:, 8 * v + t:8 * v + t + 1], m8[:, v:v + 1]
                        )

                # per-core candidates -> [KP, BLK] layout and AllGather
                pst = psP.tile([8 * KP, 128], F32, tag="t2ps", bufs=1)
                nc.tensor.transpose(pst[:], P2[:], ident[:])
                T2 = pwp.tile([8 * KP, 128], F32, tag="t2")
                nc.scalar.copy(T2[:], pst[:])
                for v in range(KP):
                    nc.sync.dma_start(
                        ag2in[v:v + 1, :].rearrange("a (t r) -> (a t) r", t=8),
                        T2[8 * v:8 * (v + 1), :],
                    )
            nc.gpsimd.collective_compute(
                "AllGather",
                mybir.AluOpType.bypass,
                replica_groups=[list(range(NCORES))],
                ins=[ag2in.opt()],
                outs=[ag2out.opt()],
            )

            # ---------------- replicated epilogue ----------------
            with (
                tc.tile_pool(name="ep", bufs=1) as ep,
                tc.tile_pool(name="psE", bufs=2, space="PSUM") as psE,
            ):
                # per-class top-k of gathered per-row candidates
                cand = ep.tile([KP, B], F32)
                nc.sync.dma_start(
                    cand[:].rearrange("v (r n) -> v r n", r=NCORES),
                    ag2out[:].rearrange("r v n -> v r n"),
                )
                m8row = ep.tile([KP, 8 * NCLS], F32)
                for c in range(NCLS):
                    s, e = int(bounds[c]), int(bounds[c + 1])
                    if e - s < 8:  # max8 needs >= 8 inputs
                        s = max(0, min(s, B - 8))
                        e = s + 8
                    nc.vector.max(m8row[:, 8 * c:8 * (c + 1)], cand[:, s:e])
                nc.sync.dma_start(bounce[:], m8row[:])
                final = ep.tile([NCLS, 8 * KP], F32)
                nc.sync.dma_start(
                    final[:].rearrange("c (v i) -> c v i", v=KP),
                    bounce[:].rearrange("v (c i) -> c v i", c=NCLS),
                )
                fm = ep.tile([NCLS, 8], F32)
                nc.vector.max(fm[:], final[:])
                fmc = ep.tile([NCLS, KP], F32)
                nc.vector.tensor_scalar_max(fmc[:], fm[:, 0:KP], 0.0)
                sqv = ep.tile([NCLS, KP], F32)
                nc.scalar.sqrt(sqv[:], fmc[:])
                ssum = ep.tile([NCLS, 1], F32)
                if KP == 1:
                    nc.vector.tensor_copy(ssum[:], sqv[:])
                else:
                    nc.vector.tensor_add(ssum[:], sqv[:, 0:1], sqv[:, 1:2])
                    for v in range(2, KP):
                        nc.vector.tensor_add(ssum[:], ssum[:], sqv[:, v:v + 1])
                rec = ep.tile([NCLS, 1], F32)
                nc.vector.reciprocal(rec[:], ssum[:])
                ones10b = ep.tile([D_OUT, 1], F32)
                nc.vector.memset(ones10b[:], 1.0)
                # partition-axis sum over the 10 classes via PE
                pli = psE.tile([1, 1], F32, tag="pli")
                nc.tensor.matmul(pli[:], ones10b[:], rec[:], start=True, stop=True)
                l_intra = ep.tile([1, 1], F32)
                nc.vector.tensor_scalar_mul(l_intra[:], pli[:], float(KP))

                # centers from gathered yT (f32 math on the small path)
                ytf = ep.tile([D_OUT, B], F32)
                for r in range(NCORES):
                    nc.sync.dma_start(
                        ytf[:, r * BLK:(r + 1) * BLK],
                        agout[r:r + 1, 0:D_OUT, :].rearrange(
                            "a p n -> (a p) n"
                        ).bitcast(F32),
                    )
                cts = ep.tile([D_OUT, NCLS], F32)
                for c in range(NCLS):
                    s, e = int(bounds[c]), int(bounds[c + 1])
                    if e > s:
                        nc.vector.reduce_sum(
                            cts[:, c:c + 1], ytf[:, s:e],
                            axis=mybir.AxisListType.X,
                        )
                    else:
                        nc.vector.memset(cts[:, c:c + 1], 0.0)
                invc = ep.tile([NCLS, NCLS], F32)
                nc.sync.dma_start(invc[:], INVC[:])
                ct = ep.tile([D_OUT, NCLS], F32)
                nc.vector.tensor_mul(ct[:], cts[:], invc[:])
                ct2 = ep.tile([D_OUT, NCLS], F32)
                nc.vector.tensor_mul(ct2[:], ct[:], ct[:])
                psq = psE.tile([1, NCLS], F32, tag="csq")
                nc.tensor.matmul(psq[:], ones10b[:], ct2[:], start=True, stop=True)
                csq = ep.tile([1, NCLS], F32)
                nc.scalar.copy(csq[:], psq[:])
                uc = ep.tile([12, NCLS], F32)
                nc.vector.tensor_scalar_mul(uc[0:10, :], ct[:], -2.0)
                nc.sync.dma_start(uc[10:11, :], CROWS[0:1, 0:NCLS].bitcast(F32))
                nc.sync.dma_start(uc[11:12, :], csq[:])
                vc = ep.tile([12, NCLS], F32)
                nc.vector.tensor_copy(vc[0:10, :], ct[:])
                nc.sync.dma_start(vc[10:11, :], csq[:])
                nc.sync.dma_start(vc[11:12, :], CROWS[0:1, 0:NCLS].bitcast(F32))
                pcd = psE.tile([NCLS, NCLS], F32, tag="cd2")
                nc.tensor.matmul(pcd[:], uc[:], vc[:], start=True, stop=True)
                ctri = ep.tile([NCLS, NCLS], F32)
                nc.sync.dma_start(ctri[:], CTRI[:])
                cde = ep.tile([NCLS, NCLS], F32)
                nc.vector.tensor_add(cde[:], pcd[:], ctri[:])
                cmin = ep.tile([NCLS, 1], F32)
                nc.vector.tensor_reduce(
                    cmin[:], cde[:], axis=mybir.AxisListType.X,
                    op=mybir.AluOpType.min,
                )
                nc.sync.dma_start(
                    cbounce[0:1, :].rearrange("a p -> p a"), cmin[:]
                )
                cminT = ep.tile([1, NCLS], F32)
                nc.sync.dma_start(cminT[:], cbounce[0:1, :])
                cm = ep.tile([1, 1], F32)
                nc.vector.tensor_reduce(
                    cm[:], cminT[:], axis=mybir.AxisListType.X,
                    op=mybir.AluOpType.min,
                )
                cmc = ep.tile([1, 1], F32)
                nc.vector.tensor_scalar_max(cmc[:], cm[:], 0.0)
                dcen = ep.tile([1, 1], F32)
                nc.scalar.sqrt(dcen[:], cmc[:])
                msc = ep.tile([1, 1], F32)
                nc.sync.dma_start(msc[:], MSC[:])
                dif = ep.tile([1, 1], F32)
                nc.vector.tensor_sub(dif[:], msc[:], dcen[:])
                linter = ep.tile([1, 1], F32)
                nc.scalar.activation(
                    linter[:], dif[:], mybir.ActivationFunctionType.Relu
                )
                asc = ep.tile([1, 1], F32)
                nc.sync.dma_start(asc[:], ASC[:])
                # loss = a*l_intra + (1-a)*l_inter = a*(l_intra-l_inter)+l_inter
                d1 = ep.tile([1, 1], F32)
                nc.vector.tensor_sub(d1[:], l_intra[:], linter[:])
                d2 = ep.tile([1, 1], F32)
                nc.vector.tensor_mul(d2[:], d1[:], asc[:])
                lossv = ep.tile([1, 1], F32)
                nc.vector.tensor_add(lossv[:], d2[:], linter[:])
                nc.sync.dma_start(LOSS[:], lossv[:])
                # debug taps
                nc.sync.dma_start(DBG[0:10, 0:10], ct[:])
                nc.sync.dma_start(DBG[0:10, 10:20], cde[:])
                nc.sync.dma_start(DBG[0:10, 20:21], cmin[:])
                nc.sync.dma_start(DBG[0:10, 21:22], ssum[:])
                nc.sync.dma_start(DBG[0:10, 22:24], fmc[:, 0:2])
                nc.sync.dma_start(DBG[0:1, 24:25], cm[:])
                nc.sync.dma_start(DBG[0:1, 25:26], dcen[:])
                nc.sync.dma_start(DBG[0:1, 26:27], linter[:])
                nc.sync.dma_start(DBG[0:1, 27:28], l_intra[:])
                nc.sync.dma_start(DBG[0:1, 28:29], csq[:, 0:1])
                nc.sync.dma_start(DBG[0:10, 29:30], cts[:, 0:10].rearrange("p (a f) -> p a f", a=1)[:, :, 0:1].rearrange("p a f -> p (a f)"))

    nc.finalize()
    return nc


def kernel(x, target, W1, b1, W2, b2, W3, b3, a, m, k):
    x = np.ascontiguousarray(np.asarray(x, dtype=np.float32))
    tgt = np.asarray(target).astype(np.int64)
    W1 = np.ascontiguousarray(np.asarray(W1, dtype=np.float32))
    b1 = np.asarray(b1, dtype=np.float32)
    W2 = np.ascontiguousarray(np.asarray(W2, dtype=np.float32))
    b2 = np.asarray(b2, dtype=np.float32)
    W3 = np.ascontiguousarray(np.asarray(W3, dtype=np.float32))
    b3 = np.asarray(b3, dtype=np.float32)
    a_v = np.asarray(a, dtype=np.float32).reshape(1, 1)
    m_v = np.asarray(m, dtype=np.float32).reshape(1, 1)
    kk = int(np.asarray(k))

    perm = np.argsort(tgt, kind="stable")
    xs = x[perm]
    ls = tgt[perm]
    counts = np.bincount(tgt, minlength=NCLS).astype(np.int64)
    assert counts.sum() == B

    key = (tuple(int(c) for c in counts), kk)
    if key not in _BUILD_CACHE:
        _BUILD_CACHE[key] = _build(key[0], kk)
    nc = _BUILD_CACHE[key]

    # host-built constants
    tri = np.where(
        np.arange(128)[None, :] > np.arange(128)[:, None], 0.0, -BIGP
    ).astype(np.float32)
    ctri = np.where(
        np.arange(NCLS)[None, :] > np.arange(NCLS)[:, None], 0.0, BIGP
    ).astype(np.float32)
    invc = np.broadcast_to(
        1.0 / np.maximum(counts, 1).astype(np.float32), (NCLS, NCLS)
    ).copy()
    ident = np.eye(128, dtype=np.float32)
    crows = np.empty((2, BLK), dtype=np.float32)
    crows[0] = 1.0
    crows[1] = -BIGP

    in_maps = []
    for c in range(NCORES):
        rows = slice(c * BLK, (c + 1) * BLK)
        oh = (ls[rows][None, :] == np.arange(NCLS)[:, None]).astype(
            np.float32
        ) * SBP
        in_maps.append(
            dict(
                X=xs[rows],
                W1T=W1, B1T=b1, W2T=W2, B2T=b2, W3T=W3, B3T=b3,
                OHB=oh, TRI=tri, CTRI=ctri, INVC=invc, IDENT=ident, CROWS=crows,
                ASC=a_v, MSC=m_v,
            )
        )

    kw = {}
    if TRACE:
        kw = dict(trace=True)
        if TRACE_CORES is not None:
            kw["trace_cores"] = TRACE_CORES
    res = run_bass_kernel_spmd(nc, in_maps, core_ids=list(range(NCORES)), **kw)
    LAST_PERF["exec_time_ns"] = res.exec_time_ns
    LAST_PERF["mean_exec_time_ns"] = res.mean_exec_time_ns
    LAST_PERF["trace"] = (
        res.instructions_and_trace[1] if res.instructions_and_trace else None
    )
    LAST_PERF["results"] = res.results
    ys = np.concatenate([r["YOUT"] for r in res.results], axis=0)
    y = np.empty_like(ys)
    y[perm] = ys
    loss = np.float32(res.results[0]["LOSS"][0, 0])
    return (y, loss)
